# revision 67
# baseline (speedup 1.0000x reference)
"""Trainium2 Bass kernel for nn_ConvblockWithTarget (dense_cnn).

Reference computation (B=4, L=32768, C=64, K=7, T=16378):
  - unfold x into windows of 2K at stride 2 -> xp[b,c,t,j] = x[b, 2t+j, c]
  - dynamic gates: wfull[b,d,l,m] = sum_c x[b, 12+l, c] * weights[d,c,m]
    gate[b,d,t,j] = wfull[b,d,2t+(j%2), j//2];  g = tanh(gate)
  - y[b,t,d] = sum_j xp * g  + skip x[b, 12+2t, d]
  - y_ = batchnorm1(y);  z = y_ @ w_conv; z = gelu_tanh(batchnorm2(z)); out = y_ + z

Sharding: 8 cores = (batch b, sequence-half h).  Each core computes
P = T//2 output positions for one batch.  Batch-norm statistics are
global -> ONE AllReduce of per-channel (sum, M-row) where
M = sum_t y y^T is the 64x64 second-moment matrix of raw y.  sumsq is
diag(M); BN2 statistics are derived analytically from M.

On-chip layout is channel-major: SBUF partitions hold (parity, channel)
for x and (seq-half-block, channel) for y.  Per 512-output chunk:
  - 7 gate matmuls with BLOCK-DIAGONAL [128,128] stationaries (both
    parities in one instruction, contraction 128) into two PSUM groups
    (taps 0-3 -> 4 banks, taps 4-6 -> 3 banks; +1 bank y accumulator
    makes exactly 8 banks).
  - 2 batched tanh instructions straight out of PSUM (scalar engine;
    the ACT engine is the phase-A bottleneck at ~3.36us/chunk).
  - 7 fp16 products on the vector engine (tensor_tensor, 2x mode).
  - window sum on the tensor engine: [I;0] skip seed + 7 [I;I] parity
    sum matmuls accumulate into PSUM.
  - y^T staged via DMA-engine transposes for the M accumulation, which
    runs interleaved with the last chunks' window matmuls (M tiles live
    in the gA/gB PSUM tag rings).
rsqrt is computed on DVE with the quake bit trick + 1 Newton step, so
the scalar engine stays on ONE table set (gelu_apprx_tanh_and_others
holds both tanh and gelu_apprx_tanh) -- no table reloads.  In BN2 the
conv bias cb cancels analytically (var2 = szq/N - (szm/N)^2,
gb = bias2 - a2*szm/N), which shortens the a2 chain that gates the
gelu pipeline.
"""

import numpy as np

K = 7


# ----------------------------------------------------------------------------
# device program
# ----------------------------------------------------------------------------

def _build_program(NSB, NTOT, P, n_cores=8, no_cc=False):
    """Emit the SPMD Bass program. NSB = superblocks (1024 outputs each) per
    core; NTOT = total valid samples across all cores (= B*T); P = valid
    outputs per core."""
    from contextlib import ExitStack

    import concourse.bacc as bacc
    import concourse.mybir as mybir
    import concourse.tile as tile
    from concourse.alu_op_type import AluOpType as alu

    dt = mybir.dt
    f32, f16, u32 = dt.float32, dt.float16, dt.uint32
    AF = mybir.ActivationFunctionType
    EPS = 1e-5
    TLOC = 1024 * NSB + 16
    FO = 512 * NSB
    NCH = 2 * NSB            # 512-output chunks
    R = n_cores

    nc = bacc.Bacc("TRN2", target_bir_lowering=False, debug=False,
                   num_devices=n_cores)

    xb_d = nc.dram_tensor("xb", [128, TLOC], f16, kind="ExternalInput")
    w2b_d = nc.dram_tensor("w2b", [128, 128 * K], f16, kind="ExternalInput")
    sid_d = nc.dram_tensor("sid", [128, 256], f16, kind="ExternalInput")
    wc32_d = nc.dram_tensor("wc32", [128, 64], f32, kind="ExternalInput")
    wc2b_d = nc.dram_tensor("wc2b", [128, 128], f16, kind="ExternalInput")
    cst_d = nc.dram_tensor("cst", [128, 8], f32, kind="ExternalInput")
    out_d = nc.dram_tensor("out", [128, FO], f16, kind="ExternalOutput")

    groups = [list(range(n_cores))]

    with ExitStack() as es:
        tc = es.enter_context(tile.TileContext(nc))
        cp = es.enter_context(tc.tile_pool(name="const", bufs=1))
        dp = es.enter_context(tc.tile_pool(name="dram", bufs=1, space="DRAM"))
        gsp = es.enter_context(tc.tile_pool(name="gsp", bufs=2))
        gep = es.enter_context(tc.tile_pool(name="gep", bufs=3))
        prp = es.enter_context(tc.tile_pool(name="prp", bufs=16))
        scp = es.enter_context(tc.tile_pool(name="scp", bufs=1))
        osb = es.enter_context(tc.tile_pool(name="osb", bufs=5))

        # persistent SBUF
        xb_sb = cp.tile([128, TLOC], f16)
        w2b_sb = cp.tile([128, 128 * K], f16)
        sid_sb = cp.tile([128, 256], f16)
        wc32_sb = cp.tile([128, 64], f32)
        wc2b_sb = cp.tile([128, 128], f16)
        cst_sb = cp.tile([128, 8], f32)
        y_sb = cp.tile([128, FO], f16)
        yT_sb = cp.tile([128, FO], f16)
        yacc = cp.tile([128, 2], f32)
        pack = cp.tile([128, 65], f32)

        # DMA issue order = criticality: first half x piece (chunk 0 only
        # needs cols <518), gate weights, rest of first piece, warm-act
        # constant, window identities, remaining x, tail constants.
        nc.sync.dma_start(out=w2b_sb[:], in_=w2b_d.ap())
        nc.sync.dma_start(out=xb_sb[:, 0:640], in_=xb_d.ap()[:, 0:640])
        nc.sync.dma_start(out=cst_sb[:], in_=cst_d.ap())
        nc.sync.dma_start(out=xb_sb[:, 640:1024], in_=xb_d.ap()[:, 640:1024])
        nc.sync.dma_start(out=sid_sb[:], in_=sid_d.ap())
        for sb in range(1, NSB):
            sl = slice(1024 * sb, 1024 * sb + 1024)
            nc.sync.dma_start(out=xb_sb[:, sl], in_=xb_d.ap()[:, sl])
        nc.sync.dma_start(out=xb_sb[:, 1024 * NSB:TLOC],
                          in_=xb_d.ap()[:, 1024 * NSB:TLOC])
        nc.sync.dma_start(out=wc32_sb[:], in_=wc32_d.ap())
        nc.sync.dma_start(out=wc2b_sb[:], in_=wc2b_d.ap())

        # PE p-state warm-up fuel that depends on no DMA: a memset tile.
        wtile = cp.tile([128, 512], f16)
        nc.vector.memset(wtile[:], 0.0)
        nc.vector.memset(pack[:], 0.0)
        zq = cp.tile([128, 1], f32)
        nc.vector.memset(zq[:], 0.0)
        ones16 = cp.tile([128, 1], f16)
        nc.vector.memset(ones16[:], 1.0)

        SI = sid_sb[:, 0:64]     # [I64; I64] window parity-sum
        SIE = sid_sb[:, 64:128]  # [I64; 0]   skip seed
        I128 = sid_sb[:, 128:256]  # full identity (PE transpose operand)
        I64 = sid_sb[0:64, 0:64]

        # Pin the ACT table set before the first tanh: a 1-element
        # Gelu_apprx_tanh forces gelu_apprx_tanh_and_others (which also
        # contains tanh), so no table reload lands mid-kernel.
        warm = scp.tile([128, 1], f32, name="actwarm")
        nc.scalar.activation(warm[:], cst_sb[:, 4:5],
                             AF.Gelu_apprx_tanh)

        def mm(out, lhsT, rhs, tp, start=True, stop=True):
            nc.tensor.matmul(out, lhsT, rhs, start=start, stop=stop,
                             tile_position=tp, skip_group_check=True)

        def quake_rsqrt(vp, rows, tagn):
            """rsqrt(vp) on DVE rows [rows] via bit trick + 1 Newton step.
            cst cols 5,6 hold uint32 constants 1 and 0x5f3759df."""
            sh = cst_sb[rows, 5:6].bitcast(u32)
            mg = cst_sb[rows, 6:7].bitcast(u32)
            i1 = scp.tile([128, 1], u32, name=f"qi1{tagn}")
            nc.vector.tensor_tensor(i1[rows, :], vp.bitcast(u32), sh,
                                    alu.logical_shift_right)
            i2 = scp.tile([128, 1], u32, name=f"qi2{tagn}")
            nc.vector.tensor_tensor(i2[rows, :], mg, i1[rows, :], alu.subtract)
            y0 = i2[rows, :].bitcast(f32)
            t = scp.tile([128, 1], f32, name=f"qt{tagn}")
            rs = scp.tile([128, 1], f32, name=f"qr{tagn}")
            nc.vector.tensor_tensor(t[rows, :], y0, y0, alu.mult)
            nc.vector.tensor_tensor(t[rows, :], t[rows, :], vp, alu.mult)
            nc.vector.tensor_scalar(t[rows, :], t[rows, :], -0.5, 1.5,
                                    alu.mult, alu.add)
            nc.vector.tensor_tensor(rs[rows, :], y0, t[rows, :], alu.mult)
            return rs

        # ------------------------------------------------------------------
        # Phase A: gates -> tanh -> products -> PE window sums -> y, y^T
        # ------------------------------------------------------------------
        with tc.tile_pool(name="psG", bufs=1, space="PSUM") as psG:
            # Ramp the PE p-state while the x/weights DMAs are in flight so
            # the first real gate matmuls run at full clock. Uses only the
            # memset tile, so it starts immediately and stays back-to-back.
            warm0 = psG.tile([128, 2048], f32, tag="gA", name="warm0")
            for wi in range(4):
                mm(warm0[:, 0:512], wtile[:, 0:128], wtile[:], (0, 0))
            y_ps = None
            pend = None  # (chunk, y_ps, prs)

            def emit_windows(ctx):
                c, yp, prs = ctx
                sb, ib = divmod(c, 2)
                base = 512 * c
                rows = yp[64 * ib:64 * ib + 64, :]
                tp = (0, 64 * ib)
                mm(rows, SIE, xb_sb[:, base + 6:base + 518], tp,
                   start=True, stop=False)
                for m in range(7):
                    mm(rows, SI, prs[m][:], tp, start=False, stop=(m == 6))
                if ib == 1:
                    ysl = slice(512 * sb, 512 * sb + 512)
                    nc.vector.tensor_copy(y_sb[:, ysl], yp[:])
                    # zero positions beyond this core's valid range: they
                    # hold the neighbouring half's data and would corrupt
                    # the global statistics.
                    jA = max(0, min(512, P - 1024 * sb))
                    jB = max(0, min(512, P - 1024 * sb - 512))
                    if jA < 512:
                        nc.gpsimd.tensor_scalar(
                            y_sb[0:64, 512 * sb + jA:512 * sb + 512],
                            y_sb[0:64, 512 * sb + jA:512 * sb + 512],
                            0.0, None, alu.mult)
                    if jB < 512:
                        nc.gpsimd.tensor_scalar(
                            y_sb[64:128, 512 * sb + jB:512 * sb + 512],
                            y_sb[64:128, 512 * sb + jB:512 * sb + 512],
                            0.0, None, alu.mult)
                    # running per-channel sum of y (last superblock's sum
                    # comes from PE ones-matmuls on trS instead, so the
                    # reduce never blocks the critical pack path)
                    if sb < NSB - 1:
                        acol = yacc[:, 0:1] if sb == 0 else yacc[:, 1:2]
                        nc.vector.tensor_reduce(
                            acol.rearrange("d (o f) -> d o f", o=1),
                            y_sb[:, ysl].rearrange("d (o f) -> d o f", o=1),
                            mybir.AxisListType.X, alu.add)
                        if sb > 0:
                            nc.vector.tensor_tensor(yacc[:, 0:1],
                                                    yacc[:, 0:1],
                                                    yacc[:, 1:2], alu.add)
                    if sb < NSB - 1:
                        nc.sync.dma_start_transpose(
                            out=yT_sb[:, ysl].rearrange("p (k d) -> p k d",
                                                        k=4),
                            in_=y_sb[:, ysl])

            for c in range(NCH):
                sb, ib = divmod(c, 2)
                base = 512 * c
                rhs_g = xb_sb[:, base + 6:base + 518]
                gA = psG.tile([128, 2048], f32, tag="gA", name=f"gA{c}")
                for m in range(4):
                    mm(gA[:, 512 * m:512 * m + 512],
                       w2b_sb[:, 128 * m:128 * m + 128], rhs_g, (0, 0))
                gB = psG.tile([128, 1536], f32, tag="gB", name=f"gB{c}")
                for m in range(4, 7):
                    mm(gB[:, 512 * (m - 4):512 * (m - 4) + 512],
                       w2b_sb[:, 128 * m:128 * m + 128], rhs_g, (0, 0))
                gsA = gsp.tile([128, 2048], f16, tag="gsA", name=f"gsA{c}")
                nc.scalar.activation(gsA[:], gA[:], AF.Tanh)
                gsB = gsp.tile([128, 1536], f16, tag="gsB", name=f"gsB{c}")
                nc.scalar.activation(gsB[:], gB[:], AF.Tanh)

                if ib == 0:
                    y_ps = psG.tile([128, 512], f32, tag="yp",
                                    name=f"yp{sb}")
                prs = []
                for m in range(7):
                    src = (gsA[:, 512 * m:512 * m + 512] if m < 4 else
                           gsB[:, 512 * (m - 4):512 * (m - 4) + 512])
                    xop = xb_sb[:, base + m:base + m + 512]
                    pr = prp.tile([128, 512], f16, tag="pr",
                                  name=f"pr{c}_{m}")
                    nc.vector.tensor_tensor(pr[:], src, xop, alu.mult)
                    prs.append(pr)

                if pend is not None:
                    emit_windows(pend)
                pend = (c, y_ps, prs)
            emit_windows(pend)

            # --------------------------------------------------------------
            # M = sum_t y y^T (both parity blocks summed directly).  Tiles
            # live in the gA/gB tag rings so they only wait on the LAST
            # chunk's tanh, not on the whole pool draining.
            # --------------------------------------------------------------
            # last superblock: PE transposes (y_sb is ready only now, and
            # the DMA-transpose round trip would sit on the critical path)
            tr_ps = psG.tile([128, 512], f16, tag="gB", name="tr_ps")
            for k in range(4):
                sl = slice(512 * (NSB - 1) + 128 * k,
                           512 * (NSB - 1) + 128 * k + 128)
                nc.tensor.transpose(tr_ps[:, 128 * k:128 * k + 128],
                                    y_sb[:, sl], I128, tile_position=(0, 0))
            trS = scp.tile([128, 512], f16, name="trS")
            nc.vector.tensor_copy(trS[:], tr_ps[:])
            # two 64-col matmuls per 128-position slice accumulate M_ee and
            # M_oo into the same [64,64] PSUM region.
            M_ps = psG.tile([128, 64], f32, tag="gA", name="M_ps")
            s_ps = psG.tile([128, 1], f32, tag="yp", name="s_ps")
            NT4 = 4 * (NSB - 1)
            for i in range(NT4):
                sl = slice(128 * i, 128 * i + 128)
                mm(M_ps[0:64, :], yT_sb[:, sl][:, 0:64],
                   yT_sb[:, sl][:, 0:64], (0, 0),
                   start=(i == 0), stop=False)
                mm(M_ps[0:64, :], yT_sb[:, sl][:, 64:128],
                   yT_sb[:, sl][:, 64:128], (0, 0),
                   start=False, stop=False)
            for k in range(4):
                tsl = slice(128 * k, 128 * k + 128)
                mm(M_ps[0:64, :], trS[:, tsl][:, 0:64], trS[:, tsl][:, 0:64],
                   (0, 0), start=False, stop=False)
                mm(M_ps[0:64, :], trS[:, tsl][:, 64:128],
                   trS[:, tsl][:, 64:128], (0, 0),
                   start=False, stop=(k == 3))
                mm(s_ps[:], trS[:, tsl], ones16[:], (0, 0),
                   start=(k == 0), stop=(k == 3))

            # pack rows 0:64 = [sum | M64]; rows 64:128 stay zero.
            # (SBUF-SBUF tensor_tensor requires equal base partitions, so
            # stage the upper half through the pack tile with a copy.)
            nc.vector.tensor_copy(pack[0:64, 0:1], yacc[64:128, 0:1])
            nc.vector.tensor_tensor(pack[0:64, 0:1], pack[0:64, 0:1],
                                    yacc[0:64, 0:1], alu.add)
            nc.vector.tensor_tensor(pack[0:64, 0:1], pack[0:64, 0:1],
                                    s_ps[0:64, :], alu.add)
            nc.vector.tensor_tensor(pack[0:64, 0:1], pack[0:64, 0:1],
                                    s_ps[64:128, :], alu.add)
            nc.vector.tensor_copy(pack[0:64, 1:65], M_ps[0:64, :])

            arin = dp.tile([64, 65], f32, name="arin")
            arout = dp.tile([64, 65], f32, name="arout",
                            addr_space="Shared")
            gat = scp.tile([128, 65], f32, name="gat")
            nc.sync.dma_start(out=arin[:], in_=pack[0:64, :])
            if n_cores == 1 or no_cc:
                # single-core variant (TimelineSim profiling) or timing
                # ablation: skip the collective (numerically wrong)
                nc.sync.dma_start(out=arout[:], in_=arin[:])
            else:
                nc.gpsimd.collective_compute(
                    "AllReduce", alu.add, replica_groups=groups,
                    ins=[arin.opt()], outs=[arout.opt()])
            nc.sync.dma_start(out=gat[0:64, :], in_=arout[:])

            h = slice(0, 64)
            tot_sum = gat[h, 0:1]
            M_tot = gat[h, 1:65]

            # ---- BN1 affine (rows 0:64; a1/b1t packed in af, dup later) ---
            af = scp.tile([128, 4], f32, name="af")  # a1 | b1t | a2 | gb
            mean1 = scp.tile([128, 1], f32)
            nc.vector.tensor_scalar(mean1[h, :], tot_sum, 1.0 / NTOT, None,
                                    alu.mult)
            d0 = scp.tile([128, 64], f32, name="d0")
            nc.vector.tensor_tensor(d0[h, :], M_tot, I64, alu.mult)
            sq = scp.tile([128, 1], f32, name="sq")
            nc.vector.tensor_reduce(
                sq[h, :].rearrange("d (o f) -> d o f", o=1),
                d0[h, :].rearrange("d (o f) -> d o f", o=1),
                mybir.AxisListType.X, alu.add)
            var0 = scp.tile([128, 1], f32)
            nc.vector.tensor_scalar(var0[h, :], sq[h, :], 1.0 / NTOT, EPS,
                                    alu.mult, alu.add)
            msq = scp.tile([128, 1], f32)
            nc.vector.tensor_tensor(msq[h, :], mean1[h, :], mean1[h, :],
                                    alu.mult)
            var1 = scp.tile([128, 1], f32)
            nc.vector.scalar_tensor_tensor(var1[h, :], msq[h, :], -1.0,
                                           var0[h, :], alu.mult, alu.add)
            rs1 = quake_rsqrt(var1[h, :], h, 1)
            nc.vector.tensor_tensor(af[h, 0:1], rs1[h, :], cst_sb[h, 0:1],
                                    alu.mult)
            tb = scp.tile([128, 1], f32)
            nc.vector.tensor_tensor(tb[h, :], mean1[h, :], af[h, 0:1],
                                    alu.mult)
            nc.vector.scalar_tensor_tensor(af[h, 1:2], tb[h, :], -1.0,
                                           cst_sb[h, 1:2], alu.mult, alu.add)
            nc.vector.tensor_copy(af[64:128, 0:2], af[0:64, 0:2])

            # ---- BN2 analytic stats (rows 0:64) --------------------------
            small_ps = psG.tile([128, 67], f32, tag="yp", name="small_ps")
            wcs32 = scp.tile([128, 64], f32)
            nc.vector.tensor_scalar(wcs32[h, :], wc32_sb[h, :], af[h, 0:1],
                                    None, alu.mult)

            # szm[d'] = wcs^T (sum_t y);  szq[d'] = diag(wcs^T M wcs).
            # With z = z_ps + cb the constant cb CANCELS from the BN2
            # normalization: var2 = szq/N - (szm/N)^2 and the gelu bias is
            # gb = bias2 - a2*szm/N, so no cb matmul is needed at all.
            z_ps0 = small_ps[:, 1:2]
            mm(z_ps0[h, :], wcs32[h, :], tot_sum, (0, 0))
            szm = scp.tile([128, 1], f32)
            nc.vector.tensor_copy(szm[h, :], z_ps0[h, :])
            V_ps = small_ps[:, 3:67]
            mm(V_ps[h, :], M_tot, wcs32[h, :], (0, 0))
            W2 = scp.tile([128, 64], f32)
            nc.vector.tensor_tensor(W2[h, :], wcs32[h, :], V_ps[h, :],
                                    alu.mult)
            q_ps = small_ps[:, 2:3]
            mm(q_ps[h, :], W2[h, :], cst_sb[h, 4:5], (0, 0))
            # conv weights pre-scaled by a1.  The +zq (a zero derived from
            # q_ps) is numerically a no-op but makes the conv burst DEPEND
            # on q_ps, so the tiny BN2 matmuls win the PE first and the
            # a2/gb chain (which gates every gelu) is never starved.
            nc.vector.tensor_scalar(zq[h, :], q_ps[h, :], 0.0, None,
                                    alu.mult)
            wcs2 = scp.tile([128, 128], f16)
            nc.vector.tensor_scalar(wcs2[:], wc2b_sb[:], af[:, 0:1],
                                    zq[:, 0:1], alu.mult, alu.add)

            # ---- BN2 affine on rows 0:64, then duplicate ----
            sm = scp.tile([128, 1], f32)
            nc.vector.tensor_scalar(sm[h, :], szm[h, :], 1.0 / NTOT, None,
                                    alu.mult)
            q2 = scp.tile([128, 1], f32)
            nc.vector.tensor_scalar(q2[h, :], q_ps[h, :], 1.0 / NTOT, EPS,
                                    alu.mult, alu.add)
            smsq = scp.tile([128, 1], f32)
            nc.vector.tensor_tensor(smsq[h, :], sm[h, :], sm[h, :], alu.mult)
            var2 = scp.tile([128, 1], f32)
            nc.vector.scalar_tensor_tensor(var2[h, :], smsq[h, :], -1.0,
                                           q2[h, :], alu.mult, alu.add)
            rs2 = quake_rsqrt(var2[h, :], h, 2)
            nc.vector.tensor_tensor(af[h, 2:3], rs2[h, :], cst_sb[h, 2:3],
                                    alu.mult)
            tg = scp.tile([128, 1], f32)
            nc.vector.tensor_tensor(tg[h, :], sm[h, :], af[h, 2:3], alu.mult)
            nc.vector.scalar_tensor_tensor(af[h, 3:4], tg[h, :], -1.0,
                                           cst_sb[h, 3:4], alu.mult, alu.add)
            nc.vector.tensor_copy(af[64:128, 2:4], af[0:64, 2:4])
            # late copy of the BN1 affine: yn reads THIS so the residual
            # normalizations queue up only after the a2/gb chain is done
            # (keeps the DVE free for the gelu-gating critical path).
            afL = scp.tile([128, 4], f32, name="afL")
            nc.vector.tensor_copy(afL[:], af[:, 0:4])

            # --------------------------------------------------------------
            # Phase B/C: 1x1 conv + gelu + BN1 residual + store
            # --------------------------------------------------------------
            spans = [(0, 512)]
            spans += [(512 + 1024 * i, 1024) for i in range(NSB // 2 - 1)]
            spans += [(512 + 1024 * (NSB // 2 - 1), 512)]
            for i, (o0, w) in enumerate(spans):
                osl = slice(o0, o0 + w)
                z_ps = psG.tile([128, 1024], f32,
                                tag=("gA" if i % 2 else "gB"),
                                name=f"z_ps{i}")
                mm(z_ps[:, 0:512], wcs2[:], y_sb[:, o0:o0 + 512], (0, 0))
                if w > 512:
                    mm(z_ps[:, 512:1024], wcs2[:],
                       y_sb[:, o0 + 512:o0 + 1024], (0, 0))
                ge = gep.tile([128, 1024], f16, tag="ge", name=f"ge{i}")
                nc.scalar.activation(ge[:, 0:w], z_ps[:, 0:w],
                                     AF.Gelu_apprx_tanh,
                                     bias=af[:, 3:4], scale=af[:, 2:3])
                yn = osb.tile([128, 1024], f16, tag="yn", name=f"yn{i}")
                nc.vector.tensor_scalar(yn[:, 0:w], y_sb[:, osl],
                                        afL[:, 0:1], afL[:, 1:2],
                                        alu.mult, alu.add)
                of = osb.tile([128, 1024], f16, tag="of", name=f"of{i}")
                nc.vector.tensor_tensor(of[:, 0:w], yn[:, 0:w], ge[:, 0:w],
                                        alu.add)
                if i < len(spans) - 1:
                    nc.sync.dma_start(out=out_d.ap()[:, osl], in_=of[:, 0:w])
                else:
                    # last span goes out on the (now idle) ACT queue so its
                    # transfer does not wait behind the SP queue's backlog.
                    nc.scalar.dma_start(out=out_d.ap()[:, osl],
                                        in_=of[:, 0:w])

    nc.compile()
    return nc


# ----------------------------------------------------------------------------
# host side
# ----------------------------------------------------------------------------

_CACHE = {}


def _get_program(NSB, NTOT, P):
    key = (NSB, NTOT, P)
    if key not in _CACHE:
        _CACHE[key] = _build_program(NSB, NTOT, P)
    return _CACHE[key]


def _prep_inputs(x, weights, w_conv, scale1, bias1, scale2, bias2):
    """Host-side layout prep. Returns (in_maps, meta)."""
    B, L, C = x.shape
    T = (L - 2 * K) // 2 + 1
    assert T % 2 == 0
    P = T // 2
    NSB = -(-P // 1024)
    TLOC = 1024 * NSB + 16
    NTOT = B * T
    LH = L // 2

    x = np.asarray(x, np.float32)
    xE = np.ascontiguousarray(x[:, 0::2, :].transpose(0, 2, 1)).astype(np.float16)
    xO = np.ascontiguousarray(x[:, 1::2, :].transpose(0, 2, 1)).astype(np.float16)

    wt = np.asarray(weights, np.float32)  # (C,C,K) = (d,c,m)
    w2b = np.zeros((128, 128 * K), np.float16)
    for m in range(K):
        blk = wt[:, :, m].T.astype(np.float16)  # [c, d]
        w2b[0:64, 128 * m:128 * m + 64] = blk
        w2b[64:128, 128 * m + 64:128 * m + 128] = blk

    sid = np.zeros((128, 256), np.float16)
    eye = np.eye(64, dtype=np.float16)
    sid[0:64, 0:64] = eye
    sid[64:128, 0:64] = eye
    sid[0:64, 64:128] = eye
    sid[:, 128:256] = np.eye(128, dtype=np.float16)

    wc32 = np.zeros((128, 64), np.float32)
    wc32[0:64] = np.asarray(w_conv, np.float32)
    wc32[64:128] = wc32[0:64]

    wc2b = np.zeros((128, 128), np.float16)
    wc2b[0:64, 0:64] = np.asarray(w_conv, np.float16)
    wc2b[64:128, 64:128] = wc2b[0:64, 0:64]

    cst = np.zeros((128, 8), np.float32)
    for i, v in enumerate([scale1, bias1, scale2, bias2]):
        v = np.asarray(v, np.float32)
        cst[0:64, i] = v
        cst[64:128, i] = v
    cst[:, 4] = 1.0
    cst_u = cst.view(np.uint32)
    cst_u[:, 5] = 1
    cst_u[:, 6] = np.uint32(0x5F3759DF)

    in_maps = []
    for core in range(2 * B):
        b, h = core // 2, core % 2
        i0 = h * P
        n = min(TLOC, LH - i0)
        xbc = np.zeros((128, TLOC), np.float16)
        xbc[0:64, :n] = xE[b, :, i0:i0 + n]
        xbc[64:128, :n] = xO[b, :, i0:i0 + n]
        in_maps.append({"xb": xbc, "w2b": w2b, "sid": sid, "wc32": wc32,
                        "wc2b": wc2b, "cst": cst})
    meta = dict(B=B, T=T, P=P, NSB=NSB, NTOT=NTOT)
    return in_maps, meta


def _assemble(results, meta, get):
    B, T, P, NSB = meta["B"], meta["T"], meta["P"], meta["NSB"]
    out = np.empty((B, T, 64), np.float32)
    for core in range(2 * B):
        b, h = core // 2, core % 2
        od = np.asarray(get(core), np.float32)  # [128, 512*NSB]
        arr = od.reshape(2, 64, NSB, 512)       # (blk, d, sb, j)
        half = arr.transpose(2, 0, 3, 1).reshape(1024 * NSB, 64)[:P]
        out[b, h * P:(h + 1) * P, :] = half
    return out


def kernel(x, weights, w_conv, scale1, bias1, scale2, bias2, _sim=False):
    in_maps, meta = _prep_inputs(x, weights, w_conv, scale1, bias1, scale2,
                                 bias2)
    nc = _get_program(meta["NSB"], meta["NTOT"], meta["P"])

    if _sim:
        from concourse.bass_interp import MultiCoreSim
        sim = MultiCoreSim(nc, num_cores=8)
        for core in range(8):
            for name, arr in in_maps[core].items():
                sim.cores[core].tensor(name)[:] = arr
        sim.simulate(check_with_hw=False)
        return _assemble(results=None, meta=meta,
                         get=lambda c: sim.cores[c].tensor("out"))

    from concourse.bass_utils import run_bass_kernel_spmd
    res = run_bass_kernel_spmd(nc, in_maps, list(range(8)))
    return _assemble(results=None, meta=meta,
                     get=lambda c: res.results[c]["out"])


if __name__ == "__main__":
    pass
